# revision 12
# baseline (speedup 1.0000x reference)
"""GRU unit kernel for Trainium2, data-parallel over 8 NeuronCores.

Computation (per batch row):
    r  = sigmoid(x @ W_i2r + b_i2r + h @ W_h2r)
    z  = sigmoid(x @ W_i2z + b_i2z + h @ W_h2z)
    h1 = tanh   (x @ W_i2h + b_i2h + r * (h @ W_h2h))
    out = (1 - z) * h1 + z * h

Sharding: batch (16384) split 8 ways; weights replicated.

Mixed precision (validated against the reference in f64/numpy sim,
rel err 1.5e-2 < 2e-2 tolerance):
  - W_i2r, W_h2r, W_h2h matmuls in fp8 e4m3 (x,h scaled by 32, W by 4096)
    using DoubleRow perf mode: two K=128 chunks per instruction -> 2x PE rate.
  - z gate (W_i2z, W_h2z) and W_i2h in bf16. W_i2h is pre-scaled by
    S = 32*4096 on the host so the fp8-scaled r*(h@W_h2h) term adds
    directly and one tanh ACT with scale=1/S descales everything.
  - biases pre-scaled on host; sigmoid/tanh descale via ACT scale imm.

Per-core structure (B_local=2048 rows = 16 m-tiles of 128):
  - weights double-buffered in SBUF across reps (loaded on the gpsimd DMA
    queue), activations streamed per m-tile as contiguous pre-blocked
    tiles (sync queue), outputs on the scalar queue.
  - per m-tile: phase order h-z(bf16), x-z + x-a(bf16), x-r(fp8),
    h-r + h-b(fp8) so pz/pa PSUM banks free early and the next m-tile's
    matmuls overlap this one's epilogue.
  - epilogue: DVE bias adds / blend, ACT sigmoid+tanh with descale.
"""

import os
import numpy as np
import ml_dtypes
from contextlib import ExitStack

import concourse.bass as bass
import concourse.tile as tile
from concourse import bacc, mybir

N_CORES = 8
B, I, H = 16384, 1024, 1024
BL = B // N_CORES           # 2048 batch rows per core
MT = BL // 128              # 16 m-tiles
KO = I // 128               # 8 k-tiles of 128 (bf16)
KP = I // 256               # 4 k-tiles of 256 (fp8 DoubleRow pairs)
F32 = mybir.dt.float32
BF16 = mybir.dt.bfloat16
F8 = mybir.dt.float8e4
BF16_NP = ml_dtypes.bfloat16
F8_NP = ml_dtypes.float8_e4m3

SX = 32.0                   # activation fp8 scale
SW = 4096.0                 # weight fp8 scale
S = SX * SW                 # psum scale of fp8 products (2^17)
DR = mybir.MatmulPerfMode.DoubleRow


def _ap_key(a):
    try:
        return (a.memref, a.offset, str(a.ap), str(a.dtype))
    except Exception:
        return ("?", id(a))


def dedupe_ldweights(nc):
    """Drop InstLdweights that reload the stationary tile already resident in
    the PE array (bacc emits one per matmul). The paired InstMatmult keeps
    both APs, so data deps survive; the removed LDW's scheduling deps are
    merged into the following instruction."""
    total_removed = 0
    for blk in nc.m.functions[0].blocks:
        insts = list(blk.instructions)
        new = []
        last_key = None
        pending = []
        for i in insts:
            t = type(i).__name__
            eng = str(getattr(i, "engine", ""))
            if t == "InstLdweights":
                key = (_ap_key(i.ins[0]), str(i.perf_mode),
                       str(i.tile_position), str(i.is_transpose))
                if key == last_key:
                    pending.append(i)
                    total_removed += 1
                    continue
                last_key = key
                new.append(i)
            else:
                if "PE" in eng and t not in ("InstMatmult",
                                             "InstEventSemaphore"):
                    last_key = None  # unknown PE inst may clobber weights
                if pending and t == "InstMatmult":
                    for j in pending:
                        i.merge_dependencies_from(j)
                    pending = []
                new.append(i)
        if pending:
            new.extend(pending)
        blk.instructions = new
    return total_removed


def build_nc(reps: int = 1):
    nc = bacc.Bacc("TRN2", target_bir_lowering=False, debug=False,
                   num_devices=N_CORES)
    AF = mybir.ActivationFunctionType

    xt16 = nc.dram_tensor("xt16", [MT * 128, KO, 128], BF16,
                          kind="ExternalInput").ap()
    ht16 = nc.dram_tensor("ht16", [MT * 128, KO, 128], BF16,
                          kind="ExternalInput").ap()
    xt8 = nc.dram_tensor("xt8", [MT * 128, KP, 2, 128], F8,
                         kind="ExternalInput").ap()
    ht8 = nc.dram_tensor("ht8", [MT * 128, KP, 2, 128], F8,
                         kind="ExternalInput").ap()
    wx16 = nc.dram_tensor("wx16", [I, 2 * H], BF16, kind="ExternalInput").ap()
    wh16 = nc.dram_tensor("wh16", [H, H], BF16, kind="ExternalInput").ap()
    wx8 = nc.dram_tensor("wx8", [I, H], F8, kind="ExternalInput").ap()
    wh8 = nc.dram_tensor("wh8", [H, 2 * H], F8, kind="ExternalInput").ap()
    biasd = nc.dram_tensor("bias", [128, 3 * H], F32,
                           kind="ExternalInput").ap()
    h16d = nc.dram_tensor("h16", [BL, H], BF16, kind="ExternalInput").ap()
    out = nc.dram_tensor("out", [BL, H], F32, kind="ExternalOutput").ap()

    wx16_r = wx16.rearrange("(ko ki) n -> ki ko n", ki=128)
    wh16_r = wh16.rearrange("(ko ki) n -> ki ko n", ki=128)
    wx8_r = wx8.rearrange("(kp two ki) n -> ki kp two n", ki=128, two=2)
    wh8_r = wh8.rearrange("(kp two ki) n -> ki kp two n", ki=128, two=2)

    with tile.TileContext(nc) as tc, ExitStack() as ctx:
        wpool = ctx.enter_context(tc.tile_pool(name="w", bufs=2))
        bpool = ctx.enter_context(tc.tile_pool(name="b", bufs=1))
        apool = ctx.enter_context(tc.tile_pool(name="a", bufs=int(os.environ.get("GRU_ABUFS", "3"))))
        hpool = ctx.enter_context(tc.tile_pool(name="h", bufs=2))
        epool = ctx.enter_context(tc.tile_pool(name="e", bufs=2))
        psum = ctx.enter_context(tc.tile_pool(name="ps", bufs=1, space="PSUM"))

        def body():
            # Weights + bias, double-buffered across reps; gpsimd DMA queue so
            # the next rep's loads don't block this rep's activation streams.
            wh16_sb = wpool.tile([128, KO, H], BF16, tag="wh16")
            wx16_sb = wpool.tile([128, KO, 2 * H], BF16, tag="wx16")
            wx8_sb = wpool.tile([128, KP, 2, H], F8, tag="wx8")
            wh8_sb = wpool.tile([128, KP, 2, 2 * H], F8, tag="wh8")
            bias_sb = bpool.tile([128, 3 * H], F32, tag="bias")
            # load order = first-use order within an m-tile
            nc.gpsimd.dma_start(wh16_sb[:], wh16_r)
            nc.gpsimd.dma_start(wx16_sb[:, :, 0:H], wx16_r[:, :, 0:H])
            nc.gpsimd.dma_start(wx16_sb[:, :, H:2 * H], wx16_r[:, :, H:2 * H])
            nc.gpsimd.dma_start(wx8_sb[:], wx8_r)
            nc.gpsimd.dma_start(wh8_sb[:, :, :, 0:H], wh8_r[:, :, :, 0:H])
            nc.gpsimd.dma_start(wh8_sb[:, :, :, H:2 * H],
                                wh8_r[:, :, :, H:2 * H])
            nc.gpsimd.dma_start(bias_sb[:], biasd)

            for mt in range(MT):
                emit_mtile(mt, wh16_sb, wx16_sb, wx8_sb, wh8_sb, bias_sb)

        def emit_mtile(mt, wh16_sb, wx16_sb, wx8_sb, wh8_sb, bias_sb):
            ms = slice(mt * 128, (mt + 1) * 128)
            # activation tiles for this m-tile (contiguous DRAM blocks)
            a_ht16 = apool.tile([128, KO, 128], BF16, tag="ht16")
            a_xt16 = apool.tile([128, KO, 128], BF16, tag="xt16")
            a_xt8 = apool.tile([128, KP, 2, 128], F8, tag="xt8")
            a_ht8 = apool.tile([128, KP, 2, 128], F8, tag="ht8")
            h16_t = hpool.tile([128, H], BF16, tag="h16")
            nc.sync.dma_start(a_ht16[:], ht16[ms, :, :])
            nc.sync.dma_start(a_xt16[:], xt16[ms, :, :])
            nc.sync.dma_start(a_xt8[:], xt8[ms, :, :, :])
            nc.sync.dma_start(a_ht8[:], ht8[ms, :, :, :])
            nc.sync.dma_start(h16_t[:], h16d[ms, :])

            ps = {}
            for g in ("r", "z", "a", "b"):
                for nh in range(2):
                    ps[(g, nh)] = psum.tile([128, 512], F32, tag=f"p{g}{nh}",
                                            name=f"p{g}{nh}")

            # phase 1: h-side z (bf16), starts pz accumulation.
            for ko in range(KO):
                for nh in range(2):
                    o = nh * 512
                    nc.tensor.matmul(ps[("z", nh)], a_ht16[:, ko, :],
                                     wh16_sb[:, ko, o:o + 512],
                                     start=(ko == 0), stop=False)
            # phase 2: x-side z + a (bf16); 4 matmuls share each stationary.
            for ko in range(KO):
                for nh in range(2):
                    o = nh * 512
                    nc.tensor.matmul(ps[("z", nh)], a_xt16[:, ko, :],
                                     wx16_sb[:, ko, o:o + 512],
                                     start=False, stop=(ko == KO - 1))
                    nc.tensor.matmul(ps[("a", nh)], a_xt16[:, ko, :],
                                     wx16_sb[:, ko, H + o:H + o + 512],
                                     start=(ko == 0), stop=(ko == KO - 1))
            # phase 3: x-side r (fp8 DoubleRow).
            fp8_reps = int(os.environ.get("GRU_FP8_REPS", "1"))
            for rr in range(fp8_reps):  # >1: timing probe only (wrong sums)
                for kp in range(KP):
                    for nh in range(2):
                        o = nh * 512
                        nc.tensor.matmul(ps[("r", nh)], a_xt8[:, kp, :, :],
                                         wx8_sb[:, kp, :, o:o + 512],
                                         start=(kp == 0 and rr == 0),
                                         stop=False, perf_mode=DR)
            # phase 4: h-side r + b (fp8 DoubleRow); 4 matmuls per stationary.
            for rr in range(fp8_reps):
                last = rr == fp8_reps - 1
                for kp in range(KP):
                    for nh in range(2):
                        o = nh * 512
                        nc.tensor.matmul(ps[("r", nh)], a_ht8[:, kp, :, :],
                                         wh8_sb[:, kp, :, o:o + 512],
                                         start=False,
                                         stop=(kp == KP - 1 and last),
                                         perf_mode=DR)
                        nc.tensor.matmul(ps[("b", nh)], a_ht8[:, kp, :, :],
                                         wh8_sb[:, kp, :, H + o:H + o + 512],
                                         start=(kp == 0 and rr == 0),
                                         stop=(kp == KP - 1 and last),
                                         perf_mode=DR)

            if os.environ.get("GRU_EPI", "full") == "lite":
                # timing probe: drain all 8 banks with single DVE reads,
                # skip the ACT/blend chain (wrong numerics)
                tl = {}
                for g in ("z", "a", "r", "b"):
                    for nh in range(2):
                        o = nh * 512
                        tl[(g, nh)] = epool.tile([128, 512], F32,
                                                 tag=f"tl{g}{nh}",
                                                 name=f"tl{g}{nh}")
                        nc.vector.tensor_add(tl[(g, nh)][:], ps[(g, nh)][:],
                                             bias_sb[:, o:o + 512])
                for nh in range(2):
                    o = nh * 512
                    nc.scalar.dma_start(out[ms, o:o + 512], tl[("b", nh)][:])
                return

            # epilogue; emission order == engine queue order. pz/pa free
            # mid-m-tile (their adds run while fp8 phases still compute);
            # the r-chain tail runs right at m-tile end.
            tz, ta, tr = {}, {}, {}
            for nh in range(2):
                o = nh * 512
                tz[nh] = epool.tile([128, 512], F32, tag="tz", name=f"tz{nh}")
                nc.vector.tensor_add(tz[nh][:], ps[("z", nh)][:],
                                     bias_sb[:, H + o:H + o + 512])
            for nh in range(2):
                nc.scalar.activation(tz[nh][:], tz[nh][:], AF.Sigmoid)
            for nh in range(2):
                o = nh * 512
                ta[nh] = epool.tile([128, 512], F32, tag="ta", name=f"ta{nh}")
                nc.vector.tensor_add(ta[nh][:], ps[("a", nh)][:],
                                     bias_sb[:, 2 * H + o:2 * H + o + 512])
            for nh in range(2):
                o = nh * 512
                tr[nh] = epool.tile([128, 512], F32, tag="tr", name=f"tr{nh}")
                nc.vector.tensor_add(tr[nh][:], ps[("r", nh)][:],
                                     bias_sb[:, o:o + 512])
            for nh in range(2):
                nc.scalar.activation(tr[nh][:], tr[nh][:], AF.Sigmoid,
                                     scale=1.0 / S)          # r
            for nh in range(2):
                nc.vector.tensor_mul(tr[nh][:], tr[nh][:], ps[("b", nh)][:])
                nc.vector.tensor_add(ta[nh][:], ta[nh][:], tr[nh][:])
            for nh in range(2):
                nc.scalar.activation(ta[nh][:], ta[nh][:], AF.Tanh,
                                     scale=1.0 / S)          # h1
            for nh in range(2):
                o = nh * 512
                nsl = slice(o, o + 512)
                nc.vector.tensor_sub(tr[nh][:], h16_t[:, nsl], ta[nh][:])
                nc.vector.tensor_mul(tr[nh][:], tz[nh][:], tr[nh][:])
                nc.vector.tensor_add(tr[nh][:], ta[nh][:], tr[nh][:])
                nc.scalar.dma_start(out[ms, nsl], tr[nh][:])

        if reps > 1:
            # two bodies per HW-loop iteration so pool double-buffering
            # actually alternates (the instruction stream is fixed per iter)
            assert reps % 2 == 0
            with tc.For_i(0, reps // 2, 1):
                body()
                body()
        else:
            body()

    nc.compile()
    if os.environ.get("GRU_DEDUP", "1") == "1":
        dedupe_ldweights(nc)
    return nc


def prep_in_maps(inputs):
    """Host-side marshalling: shard batch, transpose/cast/scale activations,
    pre-block per-m-tile activation tiles, concat weights, prescale biases."""
    g = {k: np.asarray(v) for k, v in inputs.items()}
    x, h = g["inputs"], g["hidden"]

    wx16 = np.concatenate([g["W_i2z"], g["W_i2h"] * S], axis=1)
    wx16 = np.ascontiguousarray(wx16).astype(BF16_NP)
    wh16 = np.ascontiguousarray(g["W_h2z"]).astype(BF16_NP)
    wx8 = np.ascontiguousarray(g["W_i2r"] * SW).astype(F8_NP)
    wh8 = np.concatenate([g["W_h2r"], g["W_h2h"]], axis=1) * SW
    wh8 = np.ascontiguousarray(wh8).astype(F8_NP)
    b = np.concatenate([g["b_i2r"] * S, g["b_i2z"], g["b_i2h"] * S])
    bias_b = np.ascontiguousarray(
        np.broadcast_to(b.astype(np.float32), (128, 3 * H)))

    xt16_all = x.T.astype(BF16_NP)           # [I, B]
    ht16_all = h.T.astype(BF16_NP)
    xt8_all = (x.T * SX).astype(F8_NP)
    ht8_all = (h.T * SX).astype(F8_NP)

    def blk16(a):  # [I, BL] -> [MT*ki, KO, m] contiguous
        return np.ascontiguousarray(
            a.reshape(KO, 128, MT, 128).transpose(2, 1, 0, 3)
        ).reshape(MT * 128, KO, 128)

    def blk8(a):   # [I, BL] -> [MT*ki, KP, 2, m] contiguous
        return np.ascontiguousarray(
            a.reshape(KP, 2, 128, MT, 128).transpose(3, 2, 0, 1, 4)
        ).reshape(MT * 128, KP, 2, 128)

    in_maps = []
    for c in range(N_CORES):
        sl = slice(c * BL, (c + 1) * BL)
        in_maps.append({
            "xt16": blk16(xt16_all[:, sl]),
            "ht16": blk16(ht16_all[:, sl]),
            "xt8": blk8(xt8_all[:, sl]),
            "ht8": blk8(ht8_all[:, sl]),
            "wx16": wx16,
            "wh16": wh16,
            "wx8": wx8,
            "wh8": wh8,
            "bias": bias_b,
            "h16": np.ascontiguousarray(h[sl].astype(BF16_NP)),
        })
    return in_maps


_RUNNERS = {}


def get_runner(reps: int = 1):
    """Build the bass module once and wrap it in a jitted 8-way shard_map,
    mirroring concourse.bass2jax.run_bass_via_pjrt but reusable across calls
    (so repeated executions don't re-trace/re-compile). reps>1 wraps the
    whole kernel in an on-device loop (for timing via amortization)."""
    if reps in _RUNNERS:
        return _RUNNERS[reps]
    import jax
    from jax.sharding import Mesh, PartitionSpec
    from jax.experimental.shard_map import shard_map
    from concourse.bass2jax import (_bass_exec_p, install_neuronx_cc_hook,
                                    partition_id_tensor)

    nc = build_nc(reps)
    install_neuronx_cc_hook()

    partition_name = (nc.partition_id_tensor.name
                      if nc.partition_id_tensor else None)
    in_names, out_names, out_avals, zero_outs = [], [], [], []
    for alloc in nc.m.functions[0].allocations:
        if not isinstance(alloc, mybir.MemoryLocationSet):
            continue
        name = alloc.memorylocations[0].name
        if alloc.kind == "ExternalInput":
            if name != partition_name:
                in_names.append(name)
        elif alloc.kind == "ExternalOutput":
            out_names.append(name)
            shape = tuple(alloc.tensor_shape)
            dtype = mybir.dt.np(alloc.dtype)
            out_avals.append(jax.core.ShapedArray(shape, dtype))
            zero_outs.append(np.zeros(shape, dtype))
    all_names = in_names + out_names
    if partition_name is not None:
        all_names = all_names + [partition_name]
    all_names = tuple(all_names)
    n_in, n_out = len(in_names), len(out_names)

    def _body(*args):
        operands = list(args)
        if partition_name is not None:
            operands.append(partition_id_tensor())
        outs = _bass_exec_p.bind(
            *operands,
            out_avals=tuple(out_avals),
            in_names=all_names,
            out_names=tuple(out_names),
            lowering_input_output_aliases=(),
            sim_require_finite=True,
            sim_require_nnan=True,
            nc=nc,
        )
        return tuple(outs)

    devices = jax.devices()[:N_CORES]
    mesh = Mesh(np.asarray(devices), ("core",))
    sharded = jax.jit(
        shard_map(_body, mesh=mesh,
                  in_specs=(PartitionSpec("core"),) * (n_in + n_out),
                  out_specs=(PartitionSpec("core"),) * n_out,
                  check_rep=False),
        donate_argnums=tuple(range(n_in, n_in + n_out)),
        keep_unused=True,
    )
    _RUNNERS[reps] = (sharded, in_names, out_names, zero_outs)
    return _RUNNERS[reps]


def run_on_device(in_maps):
    sharded, in_names, out_names, zero_outs = get_runner()
    concat_in = [np.concatenate([m[n] for m in in_maps], axis=0)
                 for n in in_names]
    concat_zero = [np.zeros((N_CORES * z.shape[0], *z.shape[1:]), z.dtype)
                   for z in zero_outs]
    outs = sharded(*concat_in, *concat_zero)
    return {n: np.asarray(o) for n, o in zip(out_names, outs)}


_NC = None


def kernel(**inputs):
    """Full-input entry point: shard, run on 8 NeuronCores, gather."""
    global _NC
    from concourse._compat import axon_active
    in_maps = prep_in_maps(inputs)
    if axon_active():
        return run_on_device(in_maps)["out"]
    from concourse.bass_utils import run_bass_kernel_spmd
    if _NC is None:
        _NC = build_nc(1)
    res = run_bass_kernel_spmd(_NC, in_maps, core_ids=list(range(N_CORES)))
    return np.concatenate([res.results[c]["out"] for c in range(N_CORES)],
                          axis=0)
